# revision 9
# baseline (speedup 1.0000x reference)
"""Bidirectional Mamba block on 8 Trainium2 NeuronCores.

Sharding: data-parallel over batch (8 samples -> 8 cores; the scan state
is per-sample so no collectives are needed).  Per-core layout is
feature-major: activations are [feature_partitions, t_free] tiles with
t = L = 256.

Branch 2 (the flipped-input mamba) is computed in forward time
coordinates: pointwise/matmul stages are flip-invariant, the depthwise
conv becomes anti-causal (taps t and t+1), and the selective scan runs
right-to-left via reversed free-dim access patterns, so its output needs
no un-flip.

The [L, d_inner, N]-sized selective-scan work is spread across engines:
  - ACT: dA = exp(delta * A[:,n])   (per-partition scale AP)
  - DVE: dBx = u (.) B_bc (bf16 2x mode), tensor_tensor_scan (fp32
         decay factors for accuracy), tmp = h (.) C_bc (bf16 2x)
  - PE:  sum_n tmp_n via identity-matmul PSUM accumulation, plus the
         D*xc skip term via a diag(D) matmul in the same group.
B_t/C_t rows are partition-broadcast by a DMA round trip through DRAM.
"""

import numpy as np

TRN_REPO = '/opt/trn_rl_repo'

B, L, DM = 8, 256, 512
DI, N, DTR, HID = 1024, 16, 32, 1024
EPS = 1e-5
NJ = DI // 128   # 8 d_inner blocks
NM = DM // 128   # 4 d_model blocks
NH = HID // 128  # 8 hidden blocks
T = L

_CACHE = {}


def _build_nc(R=1, debug=False):
    import sys
    if TRN_REPO not in sys.path:
        sys.path.insert(0, TRN_REPO)
    import concourse.bacc as bacc
    import concourse.mybir as mybir
    import concourse.tile as tile
    from contextlib import ExitStack

    dt = mybir.dt
    AF = mybir.ActivationFunctionType
    OP = mybir.AluOpType

    nc = bacc.Bacc("TRN2", target_bir_lowering=False, debug=False, num_devices=8)

    def din(name, shape, dty=dt.float32):
        return nc.declare_dram_parameter(name, list(shape), dty, isOutput=False)

    W = {}
    W["xT_f"] = din("xT_f", [DM, T])
    W["xT_b"] = din("xT_b", [DM, T], dt.bfloat16)
    for b in (1, 2):
        W[f"in_wT{b}"] = din(f"in_wT{b}", [DM, 2 * DI], dt.bfloat16)
        W[f"xproj_wT{b}"] = din(f"xproj_wT{b}", [DI, 64], dt.bfloat16)
        W[f"dt_wT{b}"] = din(f"dt_wT{b}", [DTR, DI], dt.bfloat16)
        W[f"out_wT{b}"] = din(f"out_wT{b}", [DI, DM], dt.bfloat16)
        W[f"diagD{b}"] = din(f"diagD{b}", [128, DI], dt.bfloat16)
        W[f"convw0{b}"] = din(f"convw0{b}", [128, NJ])
        W[f"convw1{b}"] = din(f"convw1{b}", [128, NJ])
        W[f"convb{b}"] = din(f"convb{b}", [128, NJ])
        W[f"dtb{b}"] = din(f"dtb{b}", [128, NJ])
        W[f"A{b}"] = din(f"A{b}", [128, NJ * N])
    W["pu_wT"] = din("pu_wT", [DM, HID], dt.bfloat16)
    W["pl_wT"] = din("pl_wT", [HID, DM], dt.bfloat16)
    W["pu_b"] = din("pu_b", [128, NH])
    W["pl_b"] = din("pl_b", [128, NM])
    W["ln_g"] = din("ln_g", [128, NM])
    W["ln_b"] = din("ln_b", [128, NM])
    W["ident_f"] = din("ident_f", [128, 128])
    W["ident_b"] = din("ident_b", [128, 128], dt.bfloat16)

    out_d = nc.declare_dram_parameter("out", [T, DM], dt.float32, isOutput=True)

    bc_scr = {b: nc.dram_tensor(f"bc_scr{b}", [2 * N, T], dt.bfloat16) for b in (1, 2)}
    ln_scr = nc.dram_tensor("ln_scr", [4, T], dt.float32)

    dbg = {}
    if debug:
        for nm, shape in [
            ("dbg_xc1", [DI, T]), ("dbg_delta1", [DI, T]),
            ("dbg_y1", [DI, T]), ("dbg_ys1", [128, T]),
            ("dbg_y12", [DM, T]), ("dbg_y3n", [DM, T]),
            ("dbg_xc2", [DI, T]), ("dbg_y2", [DI, T]),
        ]:
            dbg[nm] = nc.declare_dram_parameter(nm, shape, dt.float32, isOutput=True)

    with tile.TileContext(nc) as tc:
        with ExitStack() as ctx:
            consts = ctx.enter_context(tc.tile_pool(name="consts", bufs=1))
            wpool = ctx.enter_context(tc.tile_pool(name="wpool", bufs=1))
            act = ctx.enter_context(tc.tile_pool(name="act", bufs=1))
            scan_p = ctx.enter_context(tc.tile_pool(name="scanp", bufs=2))
            da_p = ctx.enter_context(tc.tile_pool(name="dap", bufs=6))
            ps = ctx.enter_context(tc.tile_pool(name="ps", bufs=1, space="PSUM"))

            def load_const(name, dty=dt.float32):
                h = consts.tile(list(W[name].shape), dty, tag=f"c_{name}", name=f"c_{name}")
                nc.sync.dma_start(h[:], W[name][:])
                return h

            cw0 = {b: load_const(f"convw0{b}") for b in (1, 2)}
            cw1 = {b: load_const(f"convw1{b}") for b in (1, 2)}
            cb = {b: load_const(f"convb{b}") for b in (1, 2)}
            dtb = {b: load_const(f"dtb{b}") for b in (1, 2)}
            Asb = {b: load_const(f"A{b}") for b in (1, 2)}
            ident_f = load_const("ident_f")
            ident_b = load_const("ident_b", dt.bfloat16)
            pu_b = load_const("pu_b")
            pl_b = load_const("pl_b")
            ln_g = load_const("ln_g")
            ln_b = load_const("ln_b")
            def load_blocks(name, nblk, tagp, dty=dt.float32, pool=None, bufs=1):
                pool = pool or consts
                rows = W[name].shape[0] // nblk
                cols = W[name].shape[1]
                ts = []
                for k in range(nblk):
                    h = pool.tile([rows, cols], dty, tag=f"{tagp}_{k}", bufs=bufs,
                                  name=f"{tagp}_{k}")
                    nc.sync.dma_start(h[:], W[name][rows * k:rows * (k + 1), :])
                    ts.append(h)
                return ts

            xTf = load_blocks("xT_f", NM, "xTf")
            xTb = load_blocks("xT_b", NM, "xTb", dt.bfloat16)

            ones_ln = consts.tile([128, 1], dt.float32, tag="ones_ln", name="ones_ln")
            nc.vector.memset(ones_ln[:], 1.0)

            def mm(out, lhsT, rhs, start, stop):
                nc.tensor.matmul(out, lhsT, rhs, start=start, stop=stop)

            for rep in range(R):
                last = rep == R - 1
                y12 = []
                XC, G = {}, {}

                # ---- stage A (both branches): in_proj + conv(tanh-silu) + gate ----
                # silu(v) = 0.5*v*(1+tanh(v/2)); the 0.5 factors are folded into
                # host-scaled xproj_w (0.5) and out_w (0.25).
                for b in (1, 2):
                    in_w = load_blocks(f"in_wT{b}", NM, "in_w", dt.bfloat16, pool=wpool)
                    xc_b, g_b = [None] * NJ, [None] * NJ
                    for j in range(NJ):
                        p = ps.tile([128, T], dt.float32, tag="mmT", bufs=3, name="p_xz")
                        for k in range(NM):
                            mm(p[:], in_w[k][:, 128 * j:128 * (j + 1)],
                               xTb[k][:], k == 0, k == NM - 1)
                        q = act.tile([128, T], dt.float32, tag="q", bufs=2, name="q")
                        nc.vector.tensor_scalar(q[:], p[:], cw1[b][:, j:j + 1],
                                                cb[b][:, j:j + 1], OP.mult, op1=OP.add)
                        if b == 1:
                            nc.vector.scalar_tensor_tensor(
                                q[:, 1:T], p[:, 0:T - 1], cw0[b][:, j:j + 1],
                                q[:, 1:T], OP.mult, OP.add)
                        else:
                            nc.vector.scalar_tensor_tensor(
                                q[:, 0:T - 1], p[:, 1:T], cw0[b][:, j:j + 1],
                                q[:, 0:T - 1], OP.mult, OP.add)
                        th = act.tile([128, T], dt.float32, tag="th", bufs=2, name="th")
                        nc.scalar.activation(th[:], q[:], AF.Tanh, scale=0.5)
                        xc_b[j] = act.tile([128, T], dt.bfloat16, tag=f"xc{b}_{j}",
                                           name=f"xc{b}_{j}")
                        nc.vector.scalar_tensor_tensor(xc_b[j][:], th[:], 1.0, q[:],
                                                       OP.add, OP.mult)
                    for j in range(NJ):
                        p = ps.tile([128, T], dt.float32, tag="mmT", bufs=3, name="p_z")
                        for k in range(NM):
                            mm(p[:], in_w[k][:, DI + 128 * j:DI + 128 * (j + 1)],
                               xTb[k][:], k == 0, k == NM - 1)
                        th = act.tile([128, T], dt.float32, tag="th", bufs=2, name="th")
                        nc.scalar.activation(th[:], p[:], AF.Tanh, scale=0.5)
                        g_b[j] = act.tile([128, T], dt.bfloat16, tag=f"g{b}_{j}",
                                          name=f"g{b}_{j}")
                        nc.vector.scalar_tensor_tensor(g_b[j][:], th[:], 1.0, p[:],
                                                       OP.add, OP.mult)
                    XC[b], G[b] = xc_b, g_b

                    if debug and last:
                        for j in range(NJ):
                            t32 = act.tile([128, T], dt.float32, tag="dbgcast", bufs=2, name="t32")
                            nc.vector.tensor_copy(t32[:], xc_b[j][:])
                            nc.sync.dma_start(dbg[f"dbg_xc{b}"][128 * j:128 * (j + 1), :], t32[:])

                for b in (1, 2):
                    xc_b, g_b = XC[b], G[b]
                    # ---- stage B: x_proj ----
                    xp_w = load_blocks(f"xproj_wT{b}", NJ, "xp_w", dt.bfloat16, pool=wpool)
                    p_dbc = ps.tile([64, T], dt.float32, tag="sm", bufs=2, name="p_dbc")
                    for j in range(NJ):
                        mm(p_dbc[:], xp_w[j][:], xc_b[j][:], j == 0, j == NJ - 1)
                    dtbc = act.tile([64, T], dt.bfloat16, tag="dtbc", name="dtbc")
                    nc.scalar.activation(dtbc[:], p_dbc[:], AF.Copy)
                    nc.sync.dma_start(bc_scr[b][:], dtbc[32:64, :])
                    Bbc = act.tile([128, N * T], dt.bfloat16, tag="Bbc", bufs=1, name="Bbc")
                    Cbc = act.tile([128, N * T], dt.bfloat16, tag="Cbc", bufs=1, name="Cbc")
                    for n in range(N):
                        nc.sync.dma_start(Bbc[:, T * n:T * (n + 1)],
                                          bc_scr[b][n:n + 1, :].to_broadcast((128, T)))
                        nc.sync.dma_start(Cbc[:, T * n:T * (n + 1)],
                                          bc_scr[b][N + n:N + n + 1, :].to_broadcast((128, T)))

                    dt_w = wpool.tile([DTR, DI], dt.bfloat16, tag="dt_w", name="dt_w")
                    nc.sync.dma_start(dt_w[:], W[f"dt_wT{b}"][:])
                    diagD = wpool.tile([128, DI], dt.bfloat16, tag="diagD", name="diagD")
                    nc.sync.dma_start(diagD[:], W[f"diagD{b}"][:])

                    # ---- stage C+D per j ----
                    y_b = [None] * NJ
                    for j in range(NJ):
                        p_d = ps.tile([128, T], dt.float32, tag="mmT", bufs=3, name="p_d")
                        mm(p_d[:], dt_w[:, 128 * j:128 * (j + 1)], dtbc[0:32, :], True, True)
                        esp = act.tile([128, T], dt.float32, tag="esp", bufs=2, name="esp")
                        nc.scalar.activation(esp[:], p_d[:], AF.Exp, bias=dtb[b][:, j:j + 1])
                        esq = act.tile([128, T], dt.float32, tag="esq", bufs=2, name="esq")
                        nc.scalar.activation(esq[:], esp[:], AF.Square)
                        delta = act.tile([128, T], dt.float32, tag="delta", bufs=3, name="delta")
                        nc.vector.scalar_tensor_tensor(delta[:], esq[:], -0.5, esp[:],
                                                       OP.mult, OP.add)
                        if debug and last and b == 1:
                            nc.sync.dma_start(dbg["dbg_delta1"][128 * j:128 * (j + 1), :], delta[:])
                        u_b = act.tile([128, T], dt.bfloat16, tag="u", bufs=3, name="u_b")
                        nc.gpsimd.tensor_tensor(u_b[:], delta[:], xc_b[j][:], OP.mult)

                        dBx = scan_p.tile([128, N * T], dt.bfloat16, tag="dBx", name="dBx")
                        nc.vector.tensor_tensor(
                            dBx[:].rearrange("p (n t) -> p n t", n=N),
                            u_b[:, None, :].to_broadcast((128, N, T)),
                            Bbc[:].rearrange("p (n t) -> p n t", n=N),
                            OP.mult)
                        h_all = scan_p.tile([128, N * T], dt.bfloat16, tag="h", name="h_all")
                        for n in range(N):
                            dA = da_p.tile([128, T], dt.float32, tag="dA", name="dA")
                            nc.scalar.activation(dA[:], delta[:], AF.Exp,
                                                 scale=Asb[b][:, N * j + n:N * j + n + 1])
                            hs = h_all[:, T * n:T * (n + 1)]
                            ds = dBx[:, T * n:T * (n + 1)]
                            if b == 1:
                                nc.vector.tensor_tensor_scan(hs, dA[:], ds, 0.0,
                                                             OP.mult, OP.add)
                            else:
                                nc.vector.tensor_tensor_scan(hs[:, ::-1], dA[:, ::-1],
                                                             ds[:, ::-1], 0.0,
                                                             OP.mult, OP.add)
                        tmp = scan_p.tile([128, N * T], dt.bfloat16, tag="tmp", bufs=1, name="tmp")
                        nc.vector.tensor_tensor(
                            tmp[:].rearrange("p (n t) -> p n t", n=N),
                            h_all[:].rearrange("p (n t) -> p n t", n=N),
                            Cbc[:].rearrange("p (n t) -> p n t", n=N),
                            OP.mult)
                        ysp = ps.tile([128, T], dt.float32, tag="ys", bufs=2, name="ysp")
                        for n in range(N):
                            mm(ysp[:], ident_b[:], tmp[:, T * n:T * (n + 1)], n == 0, False)
                        mm(ysp[:], diagD[:, 128 * j:128 * (j + 1)], xc_b[j][:], False, True)
                        if debug and last and b == 1 and j == 0:
                            t32 = act.tile([128, T], dt.float32, tag="dbgcast", bufs=2, name="t32")
                            nc.scalar.activation(t32[:], ysp[:], AF.Copy)
                            nc.sync.dma_start(dbg["dbg_ys1"][:, :], t32[:])
                        y_b[j] = act.tile([128, T], dt.bfloat16, tag=f"y_{j}", name=f"y_{j}")
                        nc.vector.tensor_tensor(y_b[j][:], ysp[:], g_b[j][:], OP.mult)
                        if debug and last:
                            t32 = act.tile([128, T], dt.float32, tag="dbgcast", bufs=2, name="t32")
                            nc.vector.tensor_copy(t32[:], y_b[j][:])
                            nc.sync.dma_start(dbg[f"dbg_y{b}"][128 * j:128 * (j + 1), :], t32[:])

                    # ---- stage E: out_proj ----
                    out_w = load_blocks(f"out_wT{b}", NJ, "out_w", dt.bfloat16, pool=wpool)
                    for m in range(NM):
                        p = ps.tile([128, T], dt.float32, tag="mmT", bufs=3, name="p_op")
                        for j in range(NJ):
                            mm(p[:], out_w[j][:, 128 * m:128 * (m + 1)],
                               y_b[j][:], j == 0, j == NJ - 1)
                        if b == 1:
                            t = act.tile([128, T], dt.float32, tag=f"y12_{m}", name=f"y12_{m}")
                            nc.vector.tensor_tensor(t[:], p[:], xTf[m][:], OP.add)
                            y12.append(t)
                        else:
                            nc.vector.tensor_tensor(y12[m][:], p[:], y12[m][:], OP.add)

                # ---- layernorm helper ----
                def layer_norm(src, scr_row, otag, want_bf):
                    mean_p = ps.tile([1, T], dt.float32, tag="sm", bufs=2, name="mean_p")
                    var_p = ps.tile([1, T], dt.float32, tag="ys", bufs=2, name="var_p")
                    for m in range(NM):
                        mm(mean_p[:], ones_ln[:], src[m][:], m == 0, m == NM - 1)
                    for m in range(NM):
                        sq = act.tile([128, T], dt.float32, tag="ln_sq", bufs=2, name="sq")
                        nc.gpsimd.tensor_tensor(sq[:], src[m][:], src[m][:], OP.mult)
                        mm(var_p[:], ones_ln[:], sq[:], m == 0, m == NM - 1)
                    mu = act.tile([1, T], dt.float32, tag="ln_mu", name="mu")
                    nc.vector.tensor_single_scalar(mu[:], mean_p[:], 1.0 / DM, OP.mult)
                    e2 = act.tile([1, T], dt.float32, tag="ln_e2", name="e2")
                    nc.vector.tensor_single_scalar(e2[:], var_p[:], 1.0 / DM, OP.mult)
                    musq = act.tile([1, T], dt.float32, tag="ln_musq", name="musq")
                    nc.vector.tensor_tensor(musq[:], mu[:], mu[:], OP.mult)
                    v = act.tile([1, T], dt.float32, tag="ln_v", name="v")
                    nc.vector.tensor_tensor(v[:], e2[:], musq[:], OP.subtract)
                    nc.vector.tensor_single_scalar(v[:], v[:], EPS, OP.add)
                    sd = act.tile([1, T], dt.float32, tag="ln_sd", name="sd")
                    nc.scalar.activation(sd[:], v[:], AF.Sqrt)
                    rstd = act.tile([1, T], dt.float32, tag="ln_rstd", name="rstd")
                    nc.vector.reciprocal(rstd[:], sd[:])
                    m2 = act.tile([1, T], dt.float32, tag="ln_m2", name="m2")
                    nc.vector.tensor_tensor(m2[:], mu[:], rstd[:], OP.mult)
                    nc.sync.dma_start(ln_scr[scr_row:scr_row + 1, :], rstd[:])
                    nc.sync.dma_start(ln_scr[scr_row + 1:scr_row + 2, :], m2[:])
                    rstd_bc = act.tile([128, T], dt.float32, tag="ln_rstd_bc", name="rstd_bc")
                    m2_bc = act.tile([128, T], dt.float32, tag="ln_m2_bc", name="m2_bc")
                    nc.sync.dma_start(rstd_bc[:],
                                      ln_scr[scr_row:scr_row + 1, :].to_broadcast((128, T)))
                    nc.sync.dma_start(m2_bc[:],
                                      ln_scr[scr_row + 1:scr_row + 2, :].to_broadcast((128, T)))
                    outs_f, outs_b = [], []
                    for m in range(NM):
                        t1 = act.tile([128, T], dt.float32, tag="ln_t1", bufs=2, name="t1")
                        nc.gpsimd.tensor_tensor(t1[:], src[m][:], rstd_bc[:], OP.mult)
                        nc.vector.tensor_tensor(t1[:], t1[:], m2_bc[:], OP.subtract)
                        of = act.tile([128, T], dt.float32, tag=f"{otag}_{m}", name=f"{otag}_{m}")
                        nc.vector.tensor_scalar(of[:], t1[:], ln_g[:, m:m + 1],
                                                ln_b[:, m:m + 1], OP.mult, op1=OP.add)
                        outs_f.append(of)
                        if want_bf:
                            ob = act.tile([128, T], dt.bfloat16, tag=f"{otag}b_{m}",
                                          name=f"{otag}b_{m}")
                            nc.vector.tensor_copy(ob[:], of[:])
                            outs_b.append(ob)
                    return outs_f, outs_b

                y3n_f, y3n_b = layer_norm(y12, 0, "y3n", True)
                if debug and last:
                    for m in range(NM):
                        nc.sync.dma_start(dbg["dbg_y12"][128 * m:128 * (m + 1), :], y12[m][:])
                        nc.sync.dma_start(dbg["dbg_y3n"][128 * m:128 * (m + 1), :], y3n_f[m][:])

                # ---- FFN ----
                pu_w = load_blocks("pu_wT", NM, "pu_w", dt.bfloat16, pool=wpool)
                pl_w = load_blocks("pl_wT", NH, "pl_w", dt.bfloat16, pool=wpool)
                hid_b = []
                for hj in range(NH):
                    p = ps.tile([128, T], dt.float32, tag="mmT", bufs=3, name="p_fh")
                    for m in range(NM):
                        mm(p[:], pu_w[m][:, 128 * hj:128 * (hj + 1)],
                           y3n_b[m][:], m == 0, m == NM - 1)
                    hb = act.tile([128, T], dt.bfloat16, tag=f"hid_{hj}", name=f"hid_{hj}")
                    nc.scalar.activation(hb[:], p[:], AF.Relu, bias=pu_b[:, hj:hj + 1])
                    hid_b.append(hb)
                y4 = []
                for m in range(NM):
                    p = ps.tile([128, T], dt.float32, tag="mmT", bufs=3, name="p_fl")
                    for hj in range(NH):
                        mm(p[:], pl_w[hj][:, 128 * m:128 * (m + 1)],
                           hid_b[hj][:], hj == 0, hj == NH - 1)
                    t4 = act.tile([128, T], dt.float32, tag=f"y4_{m}", name=f"y4_{m}")
                    nc.vector.scalar_tensor_tensor(t4[:], p[:], pl_b[:, m:m + 1],
                                                   y3n_f[m][:], OP.add, OP.add)
                    y4.append(t4)

                out_f, _ = layer_norm(y4, 2, "outf", False)

                # ---- transpose + store ----
                if last:
                    for m in range(NM):
                        for th in range(T // 128):
                            pt = ps.tile([128, 128], dt.float32, tag="sm", bufs=2, name="pt")
                            nc.tensor.transpose(pt[:], out_f[m][:, 128 * th:128 * (th + 1)],
                                                ident_f[:])
                            ot = act.tile([128, 128], dt.float32, tag="ot", name="ot")
                            nc.scalar.activation(ot[:], pt[:], AF.Copy)
                            nc.sync.dma_start(
                                out_d[128 * th:128 * (th + 1), 128 * m:128 * (m + 1)], ot[:])
    nc.compile()
    return nc


def _prep_inputs(inputs):
    import ml_dtypes
    bf16 = ml_dtypes.bfloat16
    f32 = np.float32

    def bf(a):
        return np.ascontiguousarray(np.asarray(a, f32)).astype(bf16)

    def colpack(v, nb=NJ):
        return np.ascontiguousarray(np.asarray(v, f32).reshape(nb, 128).T)

    shared = {}
    for b, pre in ((1, 'm1_'), (2, 'm2_')):
        shared[f"in_wT{b}"] = bf(np.asarray(inputs[pre + 'in_w'], f32).T)
        # 0.5: absorbs the tanh-silu half factor on xc
        shared[f"xproj_wT{b}"] = bf(0.5 * np.asarray(inputs[pre + 'xproj_w'], f32).T)
        shared[f"dt_wT{b}"] = bf(np.asarray(inputs[pre + 'dt_w'], f32).T)
        # 0.25: absorbs the half factors of both the xc and gate tanh-silus
        shared[f"out_wT{b}"] = bf(0.25 * np.asarray(inputs[pre + 'out_w'], f32).T)
        D = np.asarray(inputs[pre + 'D'], f32)
        dd = np.zeros((128, DI), f32)
        for j in range(NJ):
            dd[:, 128 * j:128 * (j + 1)] = np.diag(D[128 * j:128 * (j + 1)])
        shared[f"diagD{b}"] = dd.astype(bf16)
        cw = np.asarray(inputs[pre + 'conv_w'], f32)
        shared[f"convw0{b}"] = colpack(cw[:, 0])
        shared[f"convw1{b}"] = colpack(cw[:, 1])
        shared[f"convb{b}"] = colpack(inputs[pre + 'conv_b'])
        shared[f"dtb{b}"] = colpack(inputs[pre + 'dt_b'])
        A = -np.exp(np.asarray(inputs[pre + 'A_log'], f32))
        Ap = np.zeros((128, NJ * N), f32)
        for j in range(NJ):
            Ap[:, N * j:N * (j + 1)] = A[128 * j:128 * (j + 1), :]
        shared[f"A{b}"] = Ap
    shared["pu_wT"] = bf(np.asarray(inputs['pu_w'], f32).T)
    shared["pl_wT"] = bf(np.asarray(inputs['pl_w'], f32).T)
    shared["pu_b"] = colpack(inputs['pu_b'], NH)
    shared["pl_b"] = colpack(inputs['pl_b'], NM)
    shared["ln_g"] = colpack(inputs['ln_g'], NM)
    shared["ln_b"] = colpack(inputs['ln_b'], NM)
    shared["ident_f"] = np.eye(128, dtype=f32)
    shared["ident_b"] = np.eye(128, dtype=f32).astype(bf16)

    x = np.asarray(inputs['x'], f32)
    in_maps = []
    for i in range(B):
        m = dict(shared)
        xT = np.ascontiguousarray(x[i].T)
        m["xT_f"] = xT
        m["xT_b"] = xT.astype(bf16)
        in_maps.append(m)
    return in_maps


def kernel(**inputs):
    import sys
    if TRN_REPO not in sys.path:
        sys.path.insert(0, TRN_REPO)
    from concourse.bass_utils import run_bass_kernel_spmd

    if "nc" not in _CACHE:
        _CACHE["nc"] = _build_nc(R=1, debug=False)
    nc = _CACHE["nc"]
    in_maps = _prep_inputs(inputs)
    res = run_bass_kernel_spmd(nc, in_maps, list(range(B)))
    out = np.stack([np.asarray(res.results[i]["out"]) for i in range(B)])
    return out.astype(np.float32)


# revision 11
# speedup vs baseline: 3.3637x; 3.3637x over previous
"""Bidirectional Mamba block on 8 Trainium2 NeuronCores.

Sharding: data-parallel over batch (8 samples -> 8 cores; the scan state
is per-sample so no collectives are needed).  Per-core layout is
feature-major: activations are [feature_partitions, t_free] tiles with
t = L = 256.

Branch 2 (the flipped-input mamba) is computed in forward time
coordinates: pointwise/matmul stages are flip-invariant, the depthwise
conv becomes anti-causal (taps t and t+1), and the selective scan runs
right-to-left via reversed free-dim access patterns, so its output needs
no un-flip.

The [L, d_inner, N]-sized selective-scan work is spread across engines:
  - ACT: dA = exp(delta * A[:,n])   (per-partition scale AP)
  - DVE: dBx = u (.) B_bc (bf16 2x mode), tensor_tensor_scan (fp32
         decay factors for accuracy), tmp = h (.) C_bc (bf16 2x)
  - PE:  sum_n tmp_n via identity-matmul PSUM accumulation, plus the
         D*xc skip term via a diag(D) matmul in the same group.
B_t/C_t rows are partition-broadcast by a DMA round trip through DRAM.
"""

import numpy as np

TRN_REPO = '/opt/trn_rl_repo'

B, L, DM = 8, 256, 512
DI, N, DTR, HID = 1024, 16, 32, 1024
EPS = 1e-5
NJ = DI // 128   # 8 d_inner blocks
NM = DM // 128   # 4 d_model blocks
NH = HID // 128  # 8 hidden blocks
T = L

_CACHE = {}


def _build_nc(R=1, debug=False):
    import sys
    if TRN_REPO not in sys.path:
        sys.path.insert(0, TRN_REPO)
    import concourse.bacc as bacc
    import concourse.mybir as mybir
    import concourse.tile as tile
    from contextlib import ExitStack

    dt = mybir.dt
    AF = mybir.ActivationFunctionType
    OP = mybir.AluOpType

    nc = bacc.Bacc("TRN2", target_bir_lowering=False, debug=False, num_devices=8)

    def din(name, shape, dty=dt.float32):
        return nc.declare_dram_parameter(name, list(shape), dty, isOutput=False)

    W = {}
    W["xT_f"] = din("xT_f", [DM, T])
    W["xT_b"] = din("xT_b", [DM, T], dt.bfloat16)
    for b in (1, 2):
        W[f"in_wT{b}"] = din(f"in_wT{b}", [DM, 2 * DI], dt.bfloat16)
        W[f"xproj_wT{b}"] = din(f"xproj_wT{b}", [DI, 64], dt.bfloat16)
        W[f"dt_wT{b}"] = din(f"dt_wT{b}", [DTR, DI], dt.bfloat16)
        W[f"out_wT{b}"] = din(f"out_wT{b}", [DI, DM], dt.bfloat16)
        W[f"diagD{b}"] = din(f"diagD{b}", [128, DI], dt.bfloat16)
        W[f"convw0{b}"] = din(f"convw0{b}", [128, NJ])
        W[f"convw1{b}"] = din(f"convw1{b}", [128, NJ])
        W[f"convb{b}"] = din(f"convb{b}", [128, NJ])
        W[f"dtb{b}"] = din(f"dtb{b}", [128, NJ])
        W[f"A{b}"] = din(f"A{b}", [128, NJ * N])
    W["pu_wT"] = din("pu_wT", [DM, HID], dt.bfloat16)
    W["pl_wT"] = din("pl_wT", [HID, DM], dt.bfloat16)
    W["pu_b"] = din("pu_b", [128, NH])
    W["pl_b"] = din("pl_b", [128, NM])
    W["ln_g"] = din("ln_g", [128, NM])
    W["ln_b"] = din("ln_b", [128, NM])
    W["ident_f"] = din("ident_f", [128, 128])
    W["ident_b"] = din("ident_b", [128, 128], dt.bfloat16)

    out_d = nc.declare_dram_parameter("out", [T, DM], dt.float32, isOutput=True)

    bc_scr = {b: nc.dram_tensor(f"bc_scr{b}", [2 * N, T], dt.bfloat16) for b in (1, 2)}
    ln_scr = nc.dram_tensor("ln_scr", [4, T], dt.float32)

    dbg = {}
    if debug:
        for nm, shape in [
            ("dbg_xc1", [DI, T]), ("dbg_delta1", [DI, T]),
            ("dbg_y1", [DI, T]), ("dbg_ys1", [128, T]),
            ("dbg_y12", [DM, T]), ("dbg_y3n", [DM, T]),
            ("dbg_xc2", [DI, T]), ("dbg_y2", [DI, T]),
        ]:
            dbg[nm] = nc.declare_dram_parameter(nm, shape, dt.float32, isOutput=True)

    with tile.TileContext(nc) as tc:
        with ExitStack() as ctx:
            consts = ctx.enter_context(tc.tile_pool(name="consts", bufs=1))
            wpool = ctx.enter_context(tc.tile_pool(name="wpool", bufs=1))
            act = ctx.enter_context(tc.tile_pool(name="act", bufs=1))
            scan_p = ctx.enter_context(tc.tile_pool(name="scanp", bufs=2))
            da_p = ctx.enter_context(tc.tile_pool(name="dap", bufs=2))
            ps = ctx.enter_context(tc.tile_pool(name="ps", bufs=1, space="PSUM"))

            def load_const(name, dty=dt.float32):
                h = consts.tile(list(W[name].shape), dty, tag=f"c_{name}", name=f"c_{name}")
                nc.sync.dma_start(h[:], W[name][:])
                return h

            cw0 = {b: load_const(f"convw0{b}") for b in (1, 2)}
            cw1 = {b: load_const(f"convw1{b}") for b in (1, 2)}
            cb = {b: load_const(f"convb{b}") for b in (1, 2)}
            dtb = {b: load_const(f"dtb{b}") for b in (1, 2)}
            Asb = {b: load_const(f"A{b}") for b in (1, 2)}
            ident_f = load_const("ident_f")
            ident_b = load_const("ident_b", dt.bfloat16)
            pu_b = load_const("pu_b")
            pl_b = load_const("pl_b")
            ln_g = load_const("ln_g")
            ln_b = load_const("ln_b")
            def load_blocks(name, nblk, tagp, dty=dt.float32, pool=None, bufs=1):
                pool = pool or consts
                rows = W[name].shape[0] // nblk
                cols = W[name].shape[1]
                ts = []
                for k in range(nblk):
                    h = pool.tile([rows, cols], dty, tag=f"{tagp}_{k}", bufs=bufs,
                                  name=f"{tagp}_{k}")
                    nc.sync.dma_start(h[:], W[name][rows * k:rows * (k + 1), :])
                    ts.append(h)
                return ts

            xTf = load_blocks("xT_f", NM, "xTf")
            xTb = load_blocks("xT_b", NM, "xTb", dt.bfloat16)

            ones_ln = consts.tile([128, 1], dt.float32, tag="ones_ln", name="ones_ln")
            nc.vector.memset(ones_ln[:], 1.0)

            def mm(out, lhsT, rhs, start, stop):
                nc.tensor.matmul(out, lhsT, rhs, start=start, stop=stop)

            for rep in range(R):
                last = rep == R - 1
                y12 = []
                XC, G = {}, {}

                # ---- stage A (both branches): in_proj + conv(tanh-silu) + gate ----
                # silu(v) = 0.5*v*(1+tanh(v/2)); the 0.5 factors are folded into
                # host-scaled xproj_w (0.5) and out_w (0.25).
                for b in (1, 2):
                    in_w = load_blocks(f"in_wT{b}", NM, "in_w", dt.bfloat16, pool=wpool)
                    xc_b, g_b = [None] * NJ, [None] * NJ
                    for j in range(NJ):
                        p = ps.tile([128, T], dt.float32, tag="mmT", bufs=3, name="p_xz")
                        for k in range(NM):
                            mm(p[:], in_w[k][:, 128 * j:128 * (j + 1)],
                               xTb[k][:], k == 0, k == NM - 1)
                        q = act.tile([128, T], dt.float32, tag="q", bufs=2, name="q")
                        nc.vector.tensor_scalar(q[:], p[:], cw1[b][:, j:j + 1],
                                                cb[b][:, j:j + 1], OP.mult, op1=OP.add)
                        if b == 1:
                            nc.vector.scalar_tensor_tensor(
                                q[:, 1:T], p[:, 0:T - 1], cw0[b][:, j:j + 1],
                                q[:, 1:T], OP.mult, OP.add)
                        else:
                            nc.vector.scalar_tensor_tensor(
                                q[:, 0:T - 1], p[:, 1:T], cw0[b][:, j:j + 1],
                                q[:, 0:T - 1], OP.mult, OP.add)
                        th = act.tile([128, T], dt.float32, tag="th", bufs=2, name="th")
                        nc.scalar.activation(th[:], q[:], AF.Tanh, scale=0.5)
                        xc_b[j] = act.tile([128, T], dt.bfloat16, tag=f"xc{b}_{j}",
                                           name=f"xc{b}_{j}")
                        nc.vector.scalar_tensor_tensor(xc_b[j][:], th[:], 1.0, q[:],
                                                       OP.add, OP.mult)
                    for j in range(NJ):
                        p = ps.tile([128, T], dt.float32, tag="mmT", bufs=3, name="p_z")
                        for k in range(NM):
                            mm(p[:], in_w[k][:, DI + 128 * j:DI + 128 * (j + 1)],
                               xTb[k][:], k == 0, k == NM - 1)
                        th = act.tile([128, T], dt.float32, tag="th", bufs=2, name="th")
                        nc.scalar.activation(th[:], p[:], AF.Tanh, scale=0.5)
                        g_b[j] = act.tile([128, T], dt.bfloat16, tag=f"g{b}_{j}",
                                          name=f"g{b}_{j}")
                        nc.vector.scalar_tensor_tensor(g_b[j][:], th[:], 1.0, p[:],
                                                       OP.add, OP.mult)
                    XC[b], G[b] = xc_b, g_b

                    if debug and last:
                        for j in range(NJ):
                            t32 = act.tile([128, T], dt.float32, tag="dbgcast", bufs=2, name="t32")
                            nc.vector.tensor_copy(t32[:], xc_b[j][:])
                            nc.sync.dma_start(dbg[f"dbg_xc{b}"][128 * j:128 * (j + 1), :], t32[:])

                for b in (1, 2):
                    xc_b, g_b = XC[b], G[b]
                    # ---- stage B: x_proj ----
                    xp_w = load_blocks(f"xproj_wT{b}", NJ, "xp_w", dt.bfloat16, pool=wpool)
                    p_dbc = ps.tile([64, T], dt.float32, tag="sm", bufs=2, name="p_dbc")
                    for j in range(NJ):
                        mm(p_dbc[:], xp_w[j][:], xc_b[j][:], j == 0, j == NJ - 1)
                    dtbc = act.tile([64, T], dt.bfloat16, tag="dtbc", name="dtbc")
                    nc.scalar.activation(dtbc[:], p_dbc[:], AF.Copy)
                    nc.sync.dma_start(bc_scr[b][:], dtbc[32:64, :])
                    Bbc = act.tile([128, N * T], dt.bfloat16, tag="Bbc", bufs=1, name="Bbc")
                    Cbc = act.tile([128, N * T], dt.bfloat16, tag="Cbc", bufs=1, name="Cbc")
                    for n in range(N):
                        nc.sync.dma_start(Bbc[:, T * n:T * (n + 1)],
                                          bc_scr[b][n:n + 1, :].to_broadcast((128, T)))
                        nc.sync.dma_start(Cbc[:, T * n:T * (n + 1)],
                                          bc_scr[b][N + n:N + n + 1, :].to_broadcast((128, T)))

                    dt_w = wpool.tile([DTR, DI], dt.bfloat16, tag="dt_w", name="dt_w")
                    nc.sync.dma_start(dt_w[:], W[f"dt_wT{b}"][:])
                    diagD = wpool.tile([128, DI], dt.bfloat16, tag="diagD", name="diagD")
                    nc.sync.dma_start(diagD[:], W[f"diagD{b}"][:])

                    # ---- stage C+D per j ----
                    y_b = [None] * NJ
                    for j in range(NJ):
                        p_d = ps.tile([128, T], dt.float32, tag="mmT", bufs=3, name="p_d")
                        mm(p_d[:], dt_w[:, 128 * j:128 * (j + 1)], dtbc[0:32, :], True, True)
                        esp = act.tile([128, T], dt.float32, tag="esp", bufs=2, name="esp")
                        nc.scalar.activation(esp[:], p_d[:], AF.Exp, bias=dtb[b][:, j:j + 1])
                        esq = act.tile([128, T], dt.float32, tag="esq", bufs=2, name="esq")
                        nc.scalar.activation(esq[:], esp[:], AF.Square)
                        delta = act.tile([128, T], dt.float32, tag="delta", bufs=3, name="delta")
                        nc.vector.scalar_tensor_tensor(delta[:], esq[:], -0.5, esp[:],
                                                       OP.mult, OP.add)
                        if debug and last and b == 1:
                            nc.sync.dma_start(dbg["dbg_delta1"][128 * j:128 * (j + 1), :], delta[:])
                        u_b = act.tile([128, T], dt.bfloat16, tag="u", bufs=3, name="u_b")
                        nc.gpsimd.tensor_tensor(u_b[:], delta[:], xc_b[j][:], OP.mult)

                        dBx = scan_p.tile([128, N * T], dt.bfloat16, tag="dBx", name="dBx")
                        nc.vector.tensor_tensor(
                            dBx[:].rearrange("p (n t) -> p n t", n=N),
                            u_b[:, None, :].to_broadcast((128, N, T)),
                            Bbc[:].rearrange("p (n t) -> p n t", n=N),
                            OP.mult)
                        h_all = scan_p.tile([128, N * T], dt.bfloat16, tag="h", name="h_all")
                        # Two wide scans, 8 n-segments each: the decay factor at
                        # each segment's entry column is zeroed so no state leaks
                        # across segments.
                        NH2 = N // 2
                        for half in range(2):
                            dA = da_p.tile([128, NH2 * T], dt.float32, tag="dA",
                                           bufs=2, name="dA")
                            for ni in range(NH2):
                                n = half * NH2 + ni
                                nc.scalar.activation(
                                    dA[:, T * ni:T * (ni + 1)], delta[:], AF.Exp,
                                    scale=Asb[b][:, N * j + n:N * j + n + 1])
                            dA3 = dA[:].rearrange("p (n t) -> p n t", n=NH2)
                            hs = h_all[:, half * NH2 * T:(half + 1) * NH2 * T]
                            ds = dBx[:, half * NH2 * T:(half + 1) * NH2 * T]
                            if b == 1:
                                nc.vector.memset(dA3[:, :, 0:1], 0.0)
                                nc.vector.tensor_tensor_scan(hs, dA[:], ds, 0.0,
                                                             OP.mult, OP.add)
                            else:
                                nc.vector.memset(dA3[:, :, T - 1:T], 0.0)
                                nc.vector.tensor_tensor_scan(hs[:, ::-1], dA[:, ::-1],
                                                             ds[:, ::-1], 0.0,
                                                             OP.mult, OP.add)
                        tmp = scan_p.tile([128, N * T], dt.bfloat16, tag="tmp", bufs=1, name="tmp")
                        NS = 12 * T
                        nc.vector.tensor_tensor(
                            tmp[:, :NS].rearrange("p (n t) -> p n t", n=12),
                            h_all[:, :NS].rearrange("p (n t) -> p n t", n=12),
                            Cbc[:, :NS].rearrange("p (n t) -> p n t", n=12),
                            OP.mult)
                        nc.gpsimd.tensor_tensor(
                            tmp[:, NS:].rearrange("p (n t) -> p n t", n=4),
                            h_all[:, NS:].rearrange("p (n t) -> p n t", n=4),
                            Cbc[:, NS:].rearrange("p (n t) -> p n t", n=4),
                            OP.mult)
                        ysp = ps.tile([128, T], dt.float32, tag="ys", bufs=2, name="ysp")
                        for n in range(N):
                            mm(ysp[:], ident_b[:], tmp[:, T * n:T * (n + 1)], n == 0, False)
                        mm(ysp[:], diagD[:, 128 * j:128 * (j + 1)], xc_b[j][:], False, True)
                        if debug and last and b == 1 and j == 0:
                            t32 = act.tile([128, T], dt.float32, tag="dbgcast", bufs=2, name="t32")
                            nc.scalar.activation(t32[:], ysp[:], AF.Copy)
                            nc.sync.dma_start(dbg["dbg_ys1"][:, :], t32[:])
                        y_b[j] = act.tile([128, T], dt.bfloat16, tag=f"y_{j}", name=f"y_{j}")
                        nc.vector.tensor_tensor(y_b[j][:], ysp[:], g_b[j][:], OP.mult)
                        if debug and last:
                            t32 = act.tile([128, T], dt.float32, tag="dbgcast", bufs=2, name="t32")
                            nc.vector.tensor_copy(t32[:], y_b[j][:])
                            nc.sync.dma_start(dbg[f"dbg_y{b}"][128 * j:128 * (j + 1), :], t32[:])

                    # ---- stage E: out_proj ----
                    out_w = load_blocks(f"out_wT{b}", NJ, "out_w", dt.bfloat16, pool=wpool)
                    for m in range(NM):
                        p = ps.tile([128, T], dt.float32, tag="mmT", bufs=3, name="p_op")
                        for j in range(NJ):
                            mm(p[:], out_w[j][:, 128 * m:128 * (m + 1)],
                               y_b[j][:], j == 0, j == NJ - 1)
                        if b == 1:
                            t = act.tile([128, T], dt.float32, tag=f"y12_{m}", name=f"y12_{m}")
                            nc.vector.tensor_tensor(t[:], p[:], xTf[m][:], OP.add)
                            y12.append(t)
                        else:
                            nc.vector.tensor_tensor(y12[m][:], p[:], y12[m][:], OP.add)

                # ---- layernorm helper ----
                def layer_norm(src, scr_row, otag, want_bf):
                    mean_p = ps.tile([1, T], dt.float32, tag="sm", bufs=2, name="mean_p")
                    var_p = ps.tile([1, T], dt.float32, tag="ys", bufs=2, name="var_p")
                    for m in range(NM):
                        mm(mean_p[:], ones_ln[:], src[m][:], m == 0, m == NM - 1)
                    for m in range(NM):
                        sq = act.tile([128, T], dt.float32, tag="ln_sq", bufs=2, name="sq")
                        nc.gpsimd.tensor_tensor(sq[:], src[m][:], src[m][:], OP.mult)
                        mm(var_p[:], ones_ln[:], sq[:], m == 0, m == NM - 1)
                    mu = act.tile([1, T], dt.float32, tag="ln_mu", name="mu")
                    nc.vector.tensor_single_scalar(mu[:], mean_p[:], 1.0 / DM, OP.mult)
                    e2 = act.tile([1, T], dt.float32, tag="ln_e2", name="e2")
                    nc.vector.tensor_single_scalar(e2[:], var_p[:], 1.0 / DM, OP.mult)
                    musq = act.tile([1, T], dt.float32, tag="ln_musq", name="musq")
                    nc.vector.tensor_tensor(musq[:], mu[:], mu[:], OP.mult)
                    v = act.tile([1, T], dt.float32, tag="ln_v", name="v")
                    nc.vector.tensor_tensor(v[:], e2[:], musq[:], OP.subtract)
                    nc.vector.tensor_single_scalar(v[:], v[:], EPS, OP.add)
                    sd = act.tile([1, T], dt.float32, tag="ln_sd", name="sd")
                    nc.scalar.activation(sd[:], v[:], AF.Sqrt)
                    rstd = act.tile([1, T], dt.float32, tag="ln_rstd", name="rstd")
                    nc.vector.reciprocal(rstd[:], sd[:])
                    m2 = act.tile([1, T], dt.float32, tag="ln_m2", name="m2")
                    nc.vector.tensor_tensor(m2[:], mu[:], rstd[:], OP.mult)
                    nc.sync.dma_start(ln_scr[scr_row:scr_row + 1, :], rstd[:])
                    nc.sync.dma_start(ln_scr[scr_row + 1:scr_row + 2, :], m2[:])
                    rstd_bc = act.tile([128, T], dt.float32, tag="ln_rstd_bc", name="rstd_bc")
                    m2_bc = act.tile([128, T], dt.float32, tag="ln_m2_bc", name="m2_bc")
                    nc.sync.dma_start(rstd_bc[:],
                                      ln_scr[scr_row:scr_row + 1, :].to_broadcast((128, T)))
                    nc.sync.dma_start(m2_bc[:],
                                      ln_scr[scr_row + 1:scr_row + 2, :].to_broadcast((128, T)))
                    outs_f, outs_b = [], []
                    for m in range(NM):
                        t1 = act.tile([128, T], dt.float32, tag="ln_t1", bufs=2, name="t1")
                        nc.gpsimd.tensor_tensor(t1[:], src[m][:], rstd_bc[:], OP.mult)
                        nc.vector.tensor_tensor(t1[:], t1[:], m2_bc[:], OP.subtract)
                        of = act.tile([128, T], dt.float32, tag=f"{otag}_{m}", name=f"{otag}_{m}")
                        nc.vector.tensor_scalar(of[:], t1[:], ln_g[:, m:m + 1],
                                                ln_b[:, m:m + 1], OP.mult, op1=OP.add)
                        outs_f.append(of)
                        if want_bf:
                            ob = act.tile([128, T], dt.bfloat16, tag=f"{otag}b_{m}",
                                          name=f"{otag}b_{m}")
                            nc.vector.tensor_copy(ob[:], of[:])
                            outs_b.append(ob)
                    return outs_f, outs_b

                y3n_f, y3n_b = layer_norm(y12, 0, "y3n", True)
                if debug and last:
                    for m in range(NM):
                        nc.sync.dma_start(dbg["dbg_y12"][128 * m:128 * (m + 1), :], y12[m][:])
                        nc.sync.dma_start(dbg["dbg_y3n"][128 * m:128 * (m + 1), :], y3n_f[m][:])

                # ---- FFN ----
                pu_w = load_blocks("pu_wT", NM, "pu_w", dt.bfloat16, pool=wpool)
                pl_w = load_blocks("pl_wT", NH, "pl_w", dt.bfloat16, pool=wpool)
                hid_b = []
                for hj in range(NH):
                    p = ps.tile([128, T], dt.float32, tag="mmT", bufs=3, name="p_fh")
                    for m in range(NM):
                        mm(p[:], pu_w[m][:, 128 * hj:128 * (hj + 1)],
                           y3n_b[m][:], m == 0, m == NM - 1)
                    hb = act.tile([128, T], dt.bfloat16, tag=f"hid_{hj}", name=f"hid_{hj}")
                    nc.scalar.activation(hb[:], p[:], AF.Relu, bias=pu_b[:, hj:hj + 1])
                    hid_b.append(hb)
                y4 = []
                for m in range(NM):
                    p = ps.tile([128, T], dt.float32, tag="mmT", bufs=3, name="p_fl")
                    for hj in range(NH):
                        mm(p[:], pl_w[hj][:, 128 * m:128 * (m + 1)],
                           hid_b[hj][:], hj == 0, hj == NH - 1)
                    t4 = act.tile([128, T], dt.float32, tag=f"y4_{m}", name=f"y4_{m}")
                    nc.vector.scalar_tensor_tensor(t4[:], p[:], pl_b[:, m:m + 1],
                                                   y3n_f[m][:], OP.add, OP.add)
                    y4.append(t4)

                out_f, _ = layer_norm(y4, 2, "outf", False)

                # ---- transpose + store ----
                if last:
                    for m in range(NM):
                        for th in range(T // 128):
                            pt = ps.tile([128, 128], dt.float32, tag="sm", bufs=2, name="pt")
                            nc.tensor.transpose(pt[:], out_f[m][:, 128 * th:128 * (th + 1)],
                                                ident_f[:])
                            ot = act.tile([128, 128], dt.float32, tag="ot", name="ot")
                            nc.scalar.activation(ot[:], pt[:], AF.Copy)
                            nc.sync.dma_start(
                                out_d[128 * th:128 * (th + 1), 128 * m:128 * (m + 1)], ot[:])
    nc.compile()
    return nc


def _prep_inputs(inputs):
    import ml_dtypes
    bf16 = ml_dtypes.bfloat16
    f32 = np.float32

    def bf(a):
        return np.ascontiguousarray(np.asarray(a, f32)).astype(bf16)

    def colpack(v, nb=NJ):
        return np.ascontiguousarray(np.asarray(v, f32).reshape(nb, 128).T)

    shared = {}
    for b, pre in ((1, 'm1_'), (2, 'm2_')):
        shared[f"in_wT{b}"] = bf(np.asarray(inputs[pre + 'in_w'], f32).T)
        # 0.5: absorbs the tanh-silu half factor on xc
        shared[f"xproj_wT{b}"] = bf(0.5 * np.asarray(inputs[pre + 'xproj_w'], f32).T)
        shared[f"dt_wT{b}"] = bf(np.asarray(inputs[pre + 'dt_w'], f32).T)
        # 0.25: absorbs the half factors of both the xc and gate tanh-silus
        shared[f"out_wT{b}"] = bf(0.25 * np.asarray(inputs[pre + 'out_w'], f32).T)
        D = np.asarray(inputs[pre + 'D'], f32)
        dd = np.zeros((128, DI), f32)
        for j in range(NJ):
            dd[:, 128 * j:128 * (j + 1)] = np.diag(D[128 * j:128 * (j + 1)])
        shared[f"diagD{b}"] = dd.astype(bf16)
        cw = np.asarray(inputs[pre + 'conv_w'], f32)
        shared[f"convw0{b}"] = colpack(cw[:, 0])
        shared[f"convw1{b}"] = colpack(cw[:, 1])
        shared[f"convb{b}"] = colpack(inputs[pre + 'conv_b'])
        shared[f"dtb{b}"] = colpack(inputs[pre + 'dt_b'])
        A = -np.exp(np.asarray(inputs[pre + 'A_log'], f32))
        Ap = np.zeros((128, NJ * N), f32)
        for j in range(NJ):
            Ap[:, N * j:N * (j + 1)] = A[128 * j:128 * (j + 1), :]
        shared[f"A{b}"] = Ap
    shared["pu_wT"] = bf(np.asarray(inputs['pu_w'], f32).T)
    shared["pl_wT"] = bf(np.asarray(inputs['pl_w'], f32).T)
    shared["pu_b"] = colpack(inputs['pu_b'], NH)
    shared["pl_b"] = colpack(inputs['pl_b'], NM)
    shared["ln_g"] = colpack(inputs['ln_g'], NM)
    shared["ln_b"] = colpack(inputs['ln_b'], NM)
    shared["ident_f"] = np.eye(128, dtype=f32)
    shared["ident_b"] = np.eye(128, dtype=f32).astype(bf16)

    x = np.asarray(inputs['x'], f32)
    in_maps = []
    for i in range(B):
        m = dict(shared)
        xT = np.ascontiguousarray(x[i].T)
        m["xT_f"] = xT
        m["xT_b"] = xT.astype(bf16)
        in_maps.append(m)
    return in_maps


def kernel(**inputs):
    import sys
    if TRN_REPO not in sys.path:
        sys.path.insert(0, TRN_REPO)
    from concourse.bass_utils import run_bass_kernel_spmd

    if "nc" not in _CACHE:
        _CACHE["nc"] = _build_nc(R=1, debug=False)
    nc = _CACHE["nc"]
    in_maps = _prep_inputs(inputs)
    res = run_bass_kernel_spmd(nc, in_maps, list(range(B)))
    out = np.stack([np.asarray(res.results[i]["out"]) for i in range(B)])
    return out.astype(np.float32)
